# revision 1
# baseline (speedup 1.0000x reference)
"""BoundaryLoss kernel for 8 Trainium2 NeuronCores.

Math (equivalent to the reference):
  boundary(i,j) = [L(i,j+1) != L(i,j-1)]_edge OR [L(i+1,j) != L(i-1,j)]_edge
    (the union of class-1/class-2 indicator boundaries equals "any label
     change" because any differing pair in {0,1,2} differs in membership
     of class 1 or class 2; |gx|+|gy| > 0.1 iff either diff is nonzero)
  ce = logsumexp_c(x) - x[label]        (max-free: |x| <= ~6 so exp is safe)
  loss = sum(ce * boundary) / (sum(boundary) + 1e-8)

Sharding: pure data parallel, 4 images per core.  Each core writes
per-partition partial sums of (boundary, ce*boundary); the host sums the
16 * [128, 24] partials in float64 and does the final division.

Per-image on-chip layout: partition p holds image rows 6p..6p+5, so the
SBUF tile is [128, 6, 768] (a plain contiguous reshape - full-rate DMA).
Row-neighbour comparisons for 4 of the 6 sub-rows stay inside the
partition (adjacent t-blocks); the two seam rows per partition use small
partition-shifted SBUF->SBUF DMA copies (U5/D0) with edge clamping.
"""

import numpy as np

B, C, H, W = 32, 3, 768, 768
NCORES = 8
BLOC = B // NCORES  # images per core
P = 128
TPB = H // P        # rows per partition (6)
WCH = 2             # t-blocks per compute chunk
NCH = TPB // WCH    # chunks per image (3)
CHW = WCH * W       # columns per chunk (1536)
NACC = BLOC * NCH * 2  # accumulator columns (b-sum, cb-sum per chunk)

_CACHE = {}


def _build(label_words):
    """Build + compile the Bass module. label_words = int32 words per label
    element (2 for int64 inputs, 1 for int32)."""
    import concourse.bacc as bacc
    import concourse.tile as tile
    import concourse.mybir as mybir

    fp32 = mybir.dt.float32
    bf16 = mybir.dt.bfloat16
    i32 = mybir.dt.int32
    Alu = mybir.AluOpType
    Act = mybir.ActivationFunctionType

    nc = bacc.Bacc(
        "TRN2",
        target_bir_lowering=False,
        debug=False,
        enable_asserts=False,
        num_devices=NCORES,
    )
    preds = nc.dram_tensor(
        "preds", [BLOC, C, P, TPB * W], fp32, kind="ExternalInput"
    ).ap()
    labs = nc.dram_tensor(
        "labs", [BLOC, P, TPB * W, label_words], i32, kind="ExternalInput"
    ).ap()
    outp = nc.dram_tensor("partials", [1, 1024], fp32, kind="ExternalOutput").ap()

    with tile.TileContext(nc) as tc:
        with (
            tc.tile_pool(name="ps", bufs=1, space="PSUM") as ps_pool,
            tc.tile_pool(name="lab", bufs=2) as lab_pool,
            tc.tile_pool(name="lfb", bufs=2) as lfb_pool,
            tc.tile_pool(name="seam", bufs=2) as seam_pool,
            tc.tile_pool(name="xin", bufs=2) as x_pool,
            tc.tile_pool(name="eact", bufs=2) as e_pool,
            tc.tile_pool(name="sls", bufs=2) as s_pool,
            tc.tile_pool(name="wrk", bufs=1) as wrk,
            tc.tile_pool(name="accp", bufs=1) as accp,
        ):
            ones = accp.tile([P, 1], bf16, name="ones")
            nc.vector.memset(ones[:], 1.0)
            pb = ps_pool.tile([1, 512], fp32, name="pb")
            pcb = ps_pool.tile([1, 512], fp32, name="pcb")
            for b in range(BLOC):
                Lfb = lfb_pool.tile([P, TPB, W], bf16, name="Lfb", tag="Lfb")
                for c in range(NCH):
                    lab = lab_pool.tile([P, CHW, label_words], i32, name="lab", tag="lab")
                    nc.sync.dma_start(
                        out=lab[:], in_=labs[b, :, c * CHW : (c + 1) * CHW, :]
                    )
                    # int32 (low word) -> bf16 cast; values are exactly 0/1/2
                    nc.scalar.activation(
                        Lfb[:, 2 * c : 2 * c + 2, :], lab[:, :, 0:1], Act.Copy
                    )
                # Seam rows: U5[p] = row 6p-1 (clamped at image top),
                #            D0[p] = row 6p+6 (clamped at image bottom).
                U5 = seam_pool.tile([P, W], bf16, name="U5", tag="U5")
                D0 = seam_pool.tile([P, W], bf16, name="D0", tag="D0")
                nc.sync.dma_start(out=U5[1:P, :], in_=Lfb[0 : P - 1, TPB - 1, :])
                nc.sync.dma_start(out=U5[0:1, :], in_=Lfb[0:1, 0, :])
                nc.sync.dma_start(out=D0[0 : P - 1, :], in_=Lfb[1:P, 0, :])
                nc.sync.dma_start(out=D0[P - 1 : P, :], in_=Lfb[P - 1 : P, TPB - 1, :])
                for c in range(NCH):
                    t0 = 2 * c
                    xs = []
                    for ch in range(C):
                        x = x_pool.tile(
                            [P, WCH, W], fp32, name=f"x{ch}", tag=f"x{ch}"
                        )
                        nc.sync.dma_start(
                            out=x[:], in_=preds[b, ch, :, c * CHW : (c + 1) * CHW]
                        )
                        xs.append(x)
                    es = []
                    for ch in range(C):
                        e = e_pool.tile([P, WCH, W], bf16, name=f"e{ch}", tag=f"e{ch}")
                        nc.scalar.activation(e[:], xs[ch][:], Act.Exp)
                        es.append(e)
                    s1 = s_pool.tile([P, WCH, W], bf16, name="s1", tag="s1")
                    nc.vector.tensor_add(s1[:], es[0][:], es[1][:])
                    s2 = s_pool.tile([P, WCH, W], bf16, name="s2", tag="s2")
                    nc.vector.tensor_add(s2[:], s1[:], es[2][:])
                    lse = s_pool.tile([P, WCH, W], bf16, name="lse", tag="lse")
                    nc.scalar.activation(lse[:], s2[:], Act.Ln)

                    # x-direction label-change mask (within rows)
                    nx = wrk.tile([P, WCH, W], bf16, name="nx", tag="nx")
                    nc.vector.tensor_tensor(
                        nx[:, :, 1 : W - 1],
                        Lfb[:, t0 : t0 + 2, 0 : W - 2],
                        Lfb[:, t0 : t0 + 2, 2:W],
                        Alu.not_equal,
                    )
                    nc.vector.tensor_tensor(
                        nx[:, :, 0:1],
                        Lfb[:, t0 : t0 + 2, 0:1],
                        Lfb[:, t0 : t0 + 2, 1:2],
                        Alu.not_equal,
                    )
                    nc.vector.tensor_tensor(
                        nx[:, :, W - 1 : W],
                        Lfb[:, t0 : t0 + 2, W - 2 : W - 1],
                        Lfb[:, t0 : t0 + 2, W - 1 : W],
                        Alu.not_equal,
                    )
                    # y-direction label-change mask (across rows)
                    ny = wrk.tile([P, WCH, W], bf16, name="ny", tag="ny")
                    for j, tb in enumerate((t0, t0 + 1)):
                        in0 = U5[:] if tb == 0 else Lfb[:, tb - 1, :]
                        in1 = D0[:] if tb == TPB - 1 else Lfb[:, tb + 1, :]
                        nc.vector.tensor_tensor(
                            ny[:, j : j + 1, :], in0, in1, Alu.not_equal
                        )

                    m1 = wrk.tile([P, WCH, W], bf16, name="m1", tag="m1")
                    nc.vector.tensor_scalar(
                        m1[:], Lfb[:, t0 : t0 + 2, :], 1.0, None, Alu.is_equal
                    )
                    m2 = wrk.tile([P, WCH, W], bf16, name="m2", tag="m2")
                    nc.vector.tensor_scalar(
                        m2[:], Lfb[:, t0 : t0 + 2, :], 2.0, None, Alu.is_equal
                    )
                    # select e[label] in the exp domain via masked products
                    # (exact in bf16: e*1 and e*0 round-trip; sum of one
                    # nonzero term is exact), then ce = ln(s) - ln(e_sel)
                    m0 = wrk.tile([P, WCH, W], bf16, name="m0", tag="m0")
                    nc.vector.tensor_scalar(
                        m0[:], Lfb[:, t0 : t0 + 2, :], 0.0, None, Alu.is_equal
                    )
                    p0 = wrk.tile([P, WCH, W], bf16, name="p0", tag="p0")
                    nc.vector.tensor_mul(p0[:], m0[:], es[0][:])
                    p1 = wrk.tile([P, WCH, W], bf16, name="p1", tag="p1")
                    nc.vector.tensor_mul(p1[:], m1[:], es[1][:])
                    p2 = wrk.tile([P, WCH, W], bf16, name="p2", tag="p2")
                    nc.vector.tensor_mul(p2[:], m2[:], es[2][:])
                    s01 = wrk.tile([P, WCH, W], bf16, name="s01", tag="s01")
                    nc.vector.tensor_add(s01[:], p0[:], p1[:])
                    esel = wrk.tile([P, WCH, W], bf16, name="esel", tag="esel")
                    nc.vector.tensor_add(esel[:], s01[:], p2[:])
                    lnsel = s_pool.tile([P, WCH, W], bf16, name="lnsel", tag="lnsel")
                    nc.scalar.activation(lnsel[:], esel[:], Act.Ln)
                    ce = wrk.tile([P, WCH, W], bf16, name="ce", tag="ce")
                    nc.vector.tensor_sub(ce[:], lse[:], lnsel[:])

                    first = b == 0 and c == 0
                    last = b == BLOC - 1 and c == NCH - 1
                    bout = wrk.tile([P, CHW], bf16, name="bout", tag="bout")
                    nc.vector.tensor_tensor(bout[:], nx[:], ny[:], Alu.max)
                    cbout = wrk.tile([P, CHW], bf16, name="cbout", tag="cbout")
                    nc.vector.tensor_tensor(cbout[:], ce[:], bout[:], Alu.mult)
                    for k in range(CHW // 512):
                        nc.tensor.matmul(
                            pb[:, :],
                            ones[:],
                            bout[:, 512 * k : 512 * (k + 1)],
                            start=first and k == 0,
                            stop=last and k == CHW // 512 - 1,
                        )
                        nc.tensor.matmul(
                            pcb[:, :],
                            ones[:],
                            cbout[:, 512 * k : 512 * (k + 1)],
                            start=first and k == 0,
                            stop=last and k == CHW // 512 - 1,
                        )
            sb = accp.tile([1, 1024], fp32, name="sb")
            nc.vector.tensor_copy(sb[:, 0:512], pb[:, :])
            nc.vector.tensor_copy(sb[:, 512:1024], pcb[:, :])
            nc.sync.dma_start(out=outp[:, :], in_=sb[:])

    # Pin Exp/Ln/Copy to the one table set containing all three so the ACT
    # table loads once instead of thrashing between sets every chunk.
    from concourse import hw_specs

    KEEP = "natural_log_exp_and_others"
    orig = hw_specs.get_activation_tables

    def only_combined(arch):
        t = orig(arch)
        return {name: (funcs if name == KEEP else set()) for name, funcs in t.items()}

    patched = []
    for mod in (hw_specs, bacc):
        if getattr(mod, "get_activation_tables", None) is not None:
            patched.append((mod, mod.get_activation_tables))
            mod.get_activation_tables = only_combined
    try:
        nc.compile()
    finally:
        for mod, fn in patched:
            mod.get_activation_tables = fn
    return nc


def _get_nc(label_words):
    if label_words not in _CACHE:
        _CACHE[label_words] = _build(label_words)
    return _CACHE[label_words]


def kernel(predictions, labels):
    from concourse.bass_utils import run_bass_kernel_spmd

    preds = np.ascontiguousarray(predictions, dtype=np.float32).reshape(
        NCORES, BLOC, C, P, TPB * W
    )
    labels = np.ascontiguousarray(labels)
    if labels.dtype == np.int64:
        label_words = 2
        labs32 = labels.view("<i4")
    elif labels.dtype == np.int32:
        label_words = 1
        labs32 = labels.reshape(labels.shape + (1,))
    else:
        raise ValueError(f"unsupported labels dtype {labels.dtype}")
    labs32 = labs32.reshape(NCORES, BLOC, P, TPB * W, label_words)

    nc = _get_nc(label_words)
    in_maps = [
        {"preds": preds[i], "labs": labs32[i]} for i in range(NCORES)
    ]
    res = run_bass_kernel_spmd(nc, in_maps, list(range(NCORES))).results
    tot_b = 0.0
    tot_cb = 0.0
    for r in res:
        p = r["partials"].astype(np.float64)
        tot_b += p[0, :512].sum()
        tot_cb += p[0, 512:].sum()
    return np.float32(tot_cb / (tot_b + 1e-8))



# revision 5
# speedup vs baseline: 1.2184x; 1.2184x over previous
"""BoundaryLoss kernel for 8 Trainium2 NeuronCores.

Math (equivalent to the reference):
  boundary(i,j) = [L(i,j+1) != L(i,j-1)]_edge OR [L(i+1,j) != L(i-1,j)]_edge
    (the union of class-1/class-2 indicator boundaries equals "any label
     change" because any differing pair in {0,1,2} differs in membership
     of class 1 or class 2; |gx|+|gy| > 0.1 iff either diff is nonzero)
  ce = logsumexp_c(x) - x[label]        (max-free: |x| <= ~6 so exp is safe)
  loss = sum(ce * boundary) / (sum(boundary) + 1e-8)

Sharding: pure data parallel, 4 images per core.  Each core writes
per-column partial sums of (boundary, ce*boundary); the host sums the
8 * [1, 1024] partials in float64 and does the final division.

v2 design (vs the first working version):
  - labels arrive in SBUF already cast to bf16 via SWDGE (gpsimd) DMA
    dtype-cast; seam rows (partition-crossing row neighbours for the
    vertical gradient) are re-read straight from HBM instead of
    partition-shifted SBUF->SBUF copies (which serialized on one DMA
    queue).
  - x[label] is selected in the x domain with copy_predicated
    (select()) instead of exp-domain masked products + a second Ln:
    drops one activation pass and two DVE passes per chunk.
  - the m2 mask is computed on the scalar engine as Relu(L-1) to
    offload the vector engine (the overall bottleneck).
  - per-pixel products reduce on the tensor engine (matmul with a ones
    vector) which is otherwise idle.
"""

import numpy as np

B, C, H, W = 32, 3, 768, 768
NCORES = 8
BLOC = B // NCORES  # images per core
P = 128
TPB = H // P        # rows per partition (6)
NH = 2              # chunks (halves) per image
RPC = TPB // NH     # rows per chunk (3)
CHW = RPC * W       # columns per chunk (2304)

_CACHE = {}


def _build(label_words):
    """Build + compile the Bass module. label_words = int32 words per label
    element (2 for int64 inputs, 1 for int32)."""
    import concourse.bacc as bacc
    import concourse.tile as tile
    import concourse.mybir as mybir

    fp32 = mybir.dt.float32
    bf16 = mybir.dt.bfloat16
    i32 = mybir.dt.int32
    Alu = mybir.AluOpType
    Act = mybir.ActivationFunctionType

    nc = bacc.Bacc(
        "TRN2",
        target_bir_lowering=False,
        debug=False,
        enable_asserts=False,
        num_devices=NCORES,
    )
    preds = nc.dram_tensor(
        "preds", [BLOC, C, P, TPB * W], fp32, kind="ExternalInput"
    ).ap()
    labs = nc.dram_tensor(
        "labs", [BLOC, P, TPB * W, label_words], i32, kind="ExternalInput"
    ).ap()
    outp = nc.dram_tensor("partials", [1, 1024], fp32, kind="ExternalOutput").ap()

    with tile.TileContext(nc) as tc:
        with (
            tc.tile_pool(name="ps", bufs=1, space="PSUM") as ps_pool,
            tc.tile_pool(name="lab", bufs=2) as lab_pool,
            tc.tile_pool(name="xin", bufs=2) as x_pool,
            tc.tile_pool(name="eact", bufs=2) as e_pool,
            tc.tile_pool(name="sls", bufs=2) as s_pool,
            tc.tile_pool(name="wrk", bufs=1) as wrk,
            tc.tile_pool(name="xch", bufs=2) as xch_pool,
            tc.tile_pool(name="accp", bufs=1) as accp,
        ):
            ones = accp.tile([P, 1], bf16, name="ones")
            nc.vector.memset(ones[:], 1.0)
            negone = accp.tile([P, 1], fp32, name="negone")
            nc.vector.memset(negone[:], -1.0)
            pb = ps_pool.tile([1, 512], fp32, name="pb")
            pcb = ps_pool.tile([1, 512], fp32, name="pcb")
            SLABS = [(0, 512), (512, 1024), (1024, 1536), (1536, 2048), (2048, 2304)]
            for b in range(BLOC):
                # Lb rows: [U, r0..r5, D]; U[p] = image row 6p-1 (clamped at
                # top), D[p] = image row 6p+6 (clamped at bottom).  All loads
                # are plain affine HBM reads; the SWDGE path casts i32->bf16
                # in the DMA datapath.
                Lb = lab_pool.tile([P, TPB + 2, W], bf16, name="Lb", tag="Lb")
                nc.gpsimd.dma_start(
                    out=Lb[:, 1 : TPB + 1, :], in_=labs[b, :, :, 0:1]
                )
                nc.gpsimd.dma_start(
                    out=Lb[1:P, 0, :],
                    in_=labs[b, 0 : P - 1, (TPB - 1) * W : TPB * W, 0:1],
                )
                nc.gpsimd.dma_start(out=Lb[0:1, 0, :], in_=labs[b, 0:1, 0:W, 0:1])
                nc.gpsimd.dma_start(
                    out=Lb[0 : P - 1, TPB + 1, :], in_=labs[b, 1:P, 0:W, 0:1]
                )
                nc.gpsimd.dma_start(
                    out=Lb[P - 1 : P, TPB + 1, :],
                    in_=labs[b, P - 1 : P, (TPB - 1) * W : TPB * W, 0:1],
                )
                for h in range(NH):
                    r0 = h * RPC  # first image sub-row of this chunk
                    Lr = Lb[:, r0 + 1 : r0 + 1 + RPC, :]  # chunk label rows
                    xs = []
                    for ch in range(C):
                        x = x_pool.tile([P, CHW], fp32, name=f"x{ch}", tag=f"x{ch}")
                        nc.sync.dma_start(
                            out=x[:],
                            in_=preds[b, ch, :, h * CHW : (h + 1) * CHW],
                        )
                        xs.append(x)
                    # --- logsumexp numerator --------------------------------
                    es = []
                    for ch in range(C):
                        e = e_pool.tile([P, CHW], bf16, name=f"e{ch}", tag=f"e{ch}")
                        nc.scalar.activation(e[:], xs[ch][:], Act.Exp)
                        es.append(e)
                    s1 = wrk.tile([P, CHW], bf16, name="s1", tag="s1")
                    nc.vector.tensor_add(s1[:], es[0][:], es[1][:])
                    s2 = s_pool.tile([P, CHW], bf16, name="s2", tag="s2")
                    nc.vector.tensor_add(s2[:], s1[:], es[2][:])
                    lse = s_pool.tile([P, CHW], bf16, name="lse", tag="lse")
                    nc.scalar.activation(lse[:], s2[:], Act.Ln)

                    # --- x[label] via predicated overwrite ------------------
                    m1 = wrk.tile([P, RPC, W], bf16, name="m1", tag="m1")
                    nc.vector.tensor_scalar(m1[:], Lr, 1.0, None, Alu.is_equal)
                    m2 = s_pool.tile([P, RPC, W], bf16, name="m2", tag="m2")
                    nc.scalar.activation(m2[:], Lr, Act.Relu, bias=negone[:])
                    xsel = wrk.tile([P, CHW], bf16, name="xsel", tag="xsel")
                    nc.vector.tensor_copy(xsel[:], xs[0][:])
                    # CopyPredicated wants an integer mask; bf16 0.0/1.0
                    # bitcast to int16 is 0 / 0x3F80 — same truthiness.
                    nc.vector.copy_predicated(
                        xsel[:], m1[:].bitcast(mybir.dt.int16), xs[1][:]
                    )
                    nc.vector.copy_predicated(
                        xsel[:], m2[:].bitcast(mybir.dt.int16), xs[2][:]
                    )

                    # --- boundary mask --------------------------------------
                    nx = wrk.tile([P, RPC, W], bf16, name="nx", tag="nx")
                    nc.vector.tensor_tensor(
                        nx[:, :, 1 : W - 1],
                        Lr[:, :, 0 : W - 2],
                        Lr[:, :, 2:W],
                        Alu.not_equal,
                    )
                    nc.vector.tensor_tensor(
                        nx[:, :, 0:1], Lr[:, :, 0:1], Lr[:, :, 1:2], Alu.not_equal
                    )
                    nc.vector.tensor_tensor(
                        nx[:, :, W - 1 : W],
                        Lr[:, :, W - 2 : W - 1],
                        Lr[:, :, W - 1 : W],
                        Alu.not_equal,
                    )
                    ny = wrk.tile([P, RPC, W], bf16, name="ny", tag="ny")
                    nc.vector.tensor_tensor(
                        ny[:],
                        Lb[:, r0 : r0 + RPC, :],
                        Lb[:, r0 + 2 : r0 + 2 + RPC, :],
                        Alu.not_equal,
                    )
                    bnd = wrk.tile([P, CHW], bf16, name="bnd", tag="bnd")
                    nc.vector.tensor_tensor(bnd[:], nx[:], ny[:], Alu.max)

                    # --- weighted CE and reductions -------------------------
                    ce = wrk.tile([P, CHW], bf16, name="ce", tag="ce")
                    nc.vector.tensor_sub(ce[:], lse[:], xsel[:])
                    cb = wrk.tile([P, CHW], bf16, name="cb", tag="cb")
                    nc.vector.tensor_mul(cb[:], ce[:], bnd[:])

                    first = b == 0 and h == 0
                    last = b == BLOC - 1 and h == NH - 1
                    for k, (a0, a1) in enumerate(SLABS):
                        nc.tensor.matmul(
                            pb[:, 0 : a1 - a0],
                            ones[:],
                            bnd[:, a0:a1],
                            start=first and k == 0,
                            stop=last and k == len(SLABS) - 1,
                        )
                        nc.tensor.matmul(
                            pcb[:, 0 : a1 - a0],
                            ones[:],
                            cb[:, a0:a1],
                            start=first and k == 0,
                            stop=last and k == len(SLABS) - 1,
                        )
            sb = accp.tile([1, 1024], fp32, name="sb")
            nc.vector.tensor_copy(sb[:, 0:512], pb[:, :])
            nc.vector.tensor_copy(sb[:, 512:1024], pcb[:, :])
            nc.sync.dma_start(out=outp[:, :], in_=sb[:])

    # Pin Exp/Ln/Copy/Relu to the one table set containing all of them so the
    # ACT table loads once instead of thrashing between sets.
    from concourse import hw_specs

    KEEP = "natural_log_exp_and_others"
    orig = hw_specs.get_activation_tables

    def only_combined(arch):
        t = orig(arch)
        return {name: (funcs if name == KEEP else set()) for name, funcs in t.items()}

    patched = []
    for mod in (hw_specs, bacc):
        if getattr(mod, "get_activation_tables", None) is not None:
            patched.append((mod, mod.get_activation_tables))
            mod.get_activation_tables = only_combined
    try:
        nc.compile()
    finally:
        for mod, fn in patched:
            mod.get_activation_tables = fn
    return nc


def _get_nc(label_words):
    if label_words not in _CACHE:
        _CACHE[label_words] = _build(label_words)
    return _CACHE[label_words]


def kernel(predictions, labels):
    from concourse.bass_utils import run_bass_kernel_spmd

    preds = np.ascontiguousarray(predictions, dtype=np.float32).reshape(
        NCORES, BLOC, C, P, TPB * W
    )
    labels = np.ascontiguousarray(labels)
    if labels.dtype == np.int64:
        label_words = 2
        labs32 = labels.view("<i4")
    elif labels.dtype == np.int32:
        label_words = 1
        labs32 = labels.reshape(labels.shape + (1,))
    else:
        raise ValueError(f"unsupported labels dtype {labels.dtype}")
    labs32 = labs32.reshape(NCORES, BLOC, P, TPB * W, label_words)

    nc = _get_nc(label_words)
    in_maps = [
        {"preds": preds[i], "labs": labs32[i]} for i in range(NCORES)
    ]
    res = run_bass_kernel_spmd(nc, in_maps, list(range(NCORES))).results
    tot_b = 0.0
    tot_cb = 0.0
    for r in res:
        p = r["partials"].astype(np.float64)
        tot_b += p[0, :512].sum()
        tot_cb += p[0, 512:].sum()
    return np.float32(tot_cb / (tot_b + 1e-8))


# revision 7
# speedup vs baseline: 1.2474x; 1.0238x over previous
"""BoundaryLoss kernel for 8 Trainium2 NeuronCores.

Math (equivalent to the reference):
  boundary(i,j) = [L(i,j+1) != L(i,j-1)]_edge OR [L(i+1,j) != L(i-1,j)]_edge
    (the union of class-1/class-2 indicator boundaries equals "any label
     change" because any differing pair in {0,1,2} differs in membership
     of class 1 or class 2; |gx|+|gy| > 0.1 iff either diff is nonzero)
  ce = logsumexp_c(x) - x[label]        (max-free: |x| <= ~6 so exp is safe)
  loss = sum(ce * boundary) / (sum(boundary) + 1e-8)

Sharding: pure data parallel, 4 images per core.  Each core writes
per-column partial sums of (boundary, ce*boundary); the host sums the
8 * [1, 1024] partials in float64 and does the final division.

v2 design (vs the first working version):
  - labels arrive in SBUF already cast to bf16 via SWDGE (gpsimd) DMA
    dtype-cast; seam rows (partition-crossing row neighbours for the
    vertical gradient) are re-read straight from HBM instead of
    partition-shifted SBUF->SBUF copies (which serialized on one DMA
    queue).
  - x[label] is selected in the x domain with copy_predicated
    (select()) instead of exp-domain masked products + a second Ln:
    drops one activation pass and two DVE passes per chunk.
  - the m2 mask is computed on the scalar engine as Relu(L-1) to
    offload the vector engine (the overall bottleneck).
  - per-pixel products reduce on the tensor engine (matmul with a ones
    vector) which is otherwise idle.
"""

import numpy as np

B, C, H, W = 32, 3, 768, 768
NCORES = 8
BLOC = B // NCORES  # images per core
P = 128
TPB = H // P        # rows per partition (6)
NH = 2              # chunks (halves) per image
RPC = TPB // NH     # rows per chunk (3)
CHW = RPC * W       # columns per chunk (2304)

_CACHE = {}


def _build(label_words):
    """Build + compile the Bass module. label_words = int32 words per label
    element (2 for int64 inputs, 1 for int32)."""
    import concourse.bacc as bacc
    import concourse.tile as tile
    import concourse.mybir as mybir

    fp32 = mybir.dt.float32
    bf16 = mybir.dt.bfloat16
    i32 = mybir.dt.int32
    Alu = mybir.AluOpType
    Act = mybir.ActivationFunctionType

    nc = bacc.Bacc(
        "TRN2",
        target_bir_lowering=False,
        debug=False,
        enable_asserts=False,
        num_devices=NCORES,
    )
    preds = nc.dram_tensor(
        "preds", [BLOC, C, P, TPB * W], fp32, kind="ExternalInput"
    ).ap()
    labs = nc.dram_tensor(
        "labs", [BLOC, P, TPB * W, label_words], i32, kind="ExternalInput"
    ).ap()
    outp = nc.dram_tensor("partials", [1, 1024], fp32, kind="ExternalOutput").ap()

    with tile.TileContext(nc) as tc:
        with (
            tc.tile_pool(name="ps", bufs=1, space="PSUM") as ps_pool,
            tc.tile_pool(name="lab", bufs=2) as lab_pool,
            tc.tile_pool(name="xin", bufs=2) as x_pool,
            tc.tile_pool(name="eact", bufs=2) as e_pool,
            tc.tile_pool(name="sls", bufs=2) as s_pool,
            tc.tile_pool(name="wrk", bufs=1) as wrk,
            tc.tile_pool(name="xch", bufs=2) as xch_pool,
            tc.tile_pool(name="accp", bufs=1) as accp,
        ):
            ones = accp.tile([P, 1], bf16, name="ones")
            nc.vector.memset(ones[:], 1.0)
            negone = accp.tile([P, 1], fp32, name="negone")
            nc.vector.memset(negone[:], -1.0)
            pb = ps_pool.tile([1, 512], fp32, name="pb")
            pcb = ps_pool.tile([1, 512], fp32, name="pcb")
            SLABS = [(0, 512), (512, 1024), (1024, 1536), (1536, 2048), (2048, 2304)]
            for b in range(BLOC):
                # Lb rows: [U, r0..r5, D]; U[p] = image row 6p-1 (clamped at
                # top), D[p] = image row 6p+6 (clamped at bottom).  All loads
                # are plain affine HBM reads; the SWDGE path casts i32->bf16
                # in the DMA datapath.
                Lb = lab_pool.tile([P, TPB + 2, W], bf16, name="Lb", tag="Lb")
                nc.gpsimd.dma_start(
                    out=Lb[:, 1 : 1 + RPC, :], in_=labs[b, :, 0 : RPC * W, 0:1]
                )
                nc.gpsimd.dma_start(
                    out=Lb[:, 1 + RPC : 1 + TPB, :],
                    in_=labs[b, :, RPC * W : TPB * W, 0:1],
                )
                nc.gpsimd.dma_start(
                    out=Lb[1:P, 0, :],
                    in_=labs[b, 0 : P - 1, (TPB - 1) * W : TPB * W, 0:1],
                )
                nc.gpsimd.dma_start(out=Lb[0:1, 0, :], in_=labs[b, 0:1, 0:W, 0:1])
                nc.gpsimd.dma_start(
                    out=Lb[0 : P - 1, TPB + 1, :], in_=labs[b, 1:P, 0:W, 0:1]
                )
                nc.gpsimd.dma_start(
                    out=Lb[P - 1 : P, TPB + 1, :],
                    in_=labs[b, P - 1 : P, (TPB - 1) * W : TPB * W, 0:1],
                )
                for h in range(NH):
                    r0 = h * RPC  # first image sub-row of this chunk
                    Lr = Lb[:, r0 + 1 : r0 + 1 + RPC, :]  # chunk label rows
                    x = x_pool.tile([P, C, CHW], fp32, name="x", tag="x")
                    for ch in range(C):
                        nc.sync.dma_start(
                            out=x[:, ch, :],
                            in_=preds[b, ch, :, h * CHW : (h + 1) * CHW],
                        )
                    # --- logsumexp numerator --------------------------------
                    e = e_pool.tile([P, C, CHW], bf16, name="e", tag="e")
                    nc.scalar.activation(e[:], x[:], Act.Exp)
                    s1 = wrk.tile([P, CHW], bf16, name="s1", tag="s1")
                    nc.vector.tensor_add(s1[:], e[:, 0, :], e[:, 1, :])
                    s2 = s_pool.tile([P, CHW], bf16, name="s2", tag="s2")
                    nc.vector.tensor_add(s2[:], s1[:], e[:, 2, :])
                    lse = s_pool.tile([P, CHW], bf16, name="lse", tag="lse")
                    nc.scalar.activation(lse[:], s2[:], Act.Ln)

                    # --- x[label] via predicated overwrite ------------------
                    m1 = wrk.tile([P, RPC, W], bf16, name="m1", tag="m1")
                    nc.vector.tensor_scalar(m1[:], Lr, 1.0, None, Alu.is_equal)
                    m2 = s_pool.tile([P, RPC, W], bf16, name="m2", tag="m2")
                    nc.scalar.activation(m2[:], Lr, Act.Relu, bias=negone[:])
                    xsel = s_pool.tile([P, CHW], bf16, name="xsel", tag="xsel")
                    nc.scalar.activation(xsel[:], x[:, 0, :], Act.Copy)
                    # CopyPredicated wants an integer mask; bf16 0.0/1.0
                    # bitcast to int16 is 0 / 0x3F80 — same truthiness.
                    nc.vector.copy_predicated(
                        xsel[:], m1[:].bitcast(mybir.dt.int16), x[:, 1, :]
                    )
                    nc.vector.copy_predicated(
                        xsel[:], m2[:].bitcast(mybir.dt.int16), x[:, 2, :]
                    )

                    # --- boundary mask --------------------------------------
                    nx = wrk.tile([P, RPC, W], bf16, name="nx", tag="nx")
                    nc.vector.tensor_tensor(
                        nx[:, :, 1 : W - 1],
                        Lr[:, :, 0 : W - 2],
                        Lr[:, :, 2:W],
                        Alu.not_equal,
                    )
                    nc.vector.tensor_tensor(
                        nx[:, :, 0:1], Lr[:, :, 0:1], Lr[:, :, 1:2], Alu.not_equal
                    )
                    nc.vector.tensor_tensor(
                        nx[:, :, W - 1 : W],
                        Lr[:, :, W - 2 : W - 1],
                        Lr[:, :, W - 1 : W],
                        Alu.not_equal,
                    )
                    ny = wrk.tile([P, RPC, W], bf16, name="ny", tag="ny")
                    nc.vector.tensor_tensor(
                        ny[:],
                        Lb[:, r0 : r0 + RPC, :],
                        Lb[:, r0 + 2 : r0 + 2 + RPC, :],
                        Alu.not_equal,
                    )
                    bnd = wrk.tile([P, CHW], bf16, name="bnd", tag="bnd")
                    nc.vector.tensor_tensor(bnd[:], nx[:], ny[:], Alu.max)

                    # --- weighted CE and reductions -------------------------
                    ce = wrk.tile([P, CHW], bf16, name="ce", tag="ce")
                    nc.vector.tensor_sub(ce[:], lse[:], xsel[:])
                    cb = wrk.tile([P, CHW], bf16, name="cb", tag="cb")
                    nc.vector.tensor_mul(cb[:], ce[:], bnd[:])

                    first = b == 0 and h == 0
                    last = b == BLOC - 1 and h == NH - 1
                    for k, (a0, a1) in enumerate(SLABS):
                        nc.tensor.matmul(
                            pb[:, 0 : a1 - a0],
                            ones[:],
                            bnd[:, a0:a1],
                            start=first and k == 0,
                            stop=last and k == len(SLABS) - 1,
                        )
                        nc.tensor.matmul(
                            pcb[:, 0 : a1 - a0],
                            ones[:],
                            cb[:, a0:a1],
                            start=first and k == 0,
                            stop=last and k == len(SLABS) - 1,
                        )
            sb = accp.tile([1, 1024], fp32, name="sb")
            nc.vector.tensor_copy(sb[:, 0:512], pb[:, :])
            nc.vector.tensor_copy(sb[:, 512:1024], pcb[:, :])
            nc.sync.dma_start(out=outp[:, :], in_=sb[:])

    # Pin Exp/Ln/Copy/Relu to the one table set containing all of them so the
    # ACT table loads once instead of thrashing between sets.
    from concourse import hw_specs

    KEEP = "natural_log_exp_and_others"
    orig = hw_specs.get_activation_tables

    def only_combined(arch):
        t = orig(arch)
        return {name: (funcs if name == KEEP else set()) for name, funcs in t.items()}

    patched = []
    for mod in (hw_specs, bacc):
        if getattr(mod, "get_activation_tables", None) is not None:
            patched.append((mod, mod.get_activation_tables))
            mod.get_activation_tables = only_combined
    try:
        nc.compile()
    finally:
        for mod, fn in patched:
            mod.get_activation_tables = fn
    return nc


def _get_nc(label_words):
    if label_words not in _CACHE:
        _CACHE[label_words] = _build(label_words)
    return _CACHE[label_words]


def kernel(predictions, labels):
    from concourse.bass_utils import run_bass_kernel_spmd

    preds = np.ascontiguousarray(predictions, dtype=np.float32).reshape(
        NCORES, BLOC, C, P, TPB * W
    )
    labels = np.ascontiguousarray(labels)
    if labels.dtype == np.int64:
        label_words = 2
        labs32 = labels.view("<i4")
    elif labels.dtype == np.int32:
        label_words = 1
        labs32 = labels.reshape(labels.shape + (1,))
    else:
        raise ValueError(f"unsupported labels dtype {labels.dtype}")
    labs32 = labs32.reshape(NCORES, BLOC, P, TPB * W, label_words)

    nc = _get_nc(label_words)
    in_maps = [
        {"preds": preds[i], "labs": labs32[i]} for i in range(NCORES)
    ]
    res = run_bass_kernel_spmd(nc, in_maps, list(range(NCORES))).results
    tot_b = 0.0
    tot_cb = 0.0
    for r in res:
        p = r["partials"].astype(np.float64)
        tot_b += p[0, :512].sum()
        tot_cb += p[0, 512:].sum()
    return np.float32(tot_cb / (tot_b + 1e-8))
